# revision 14
# baseline (speedup 1.0000x reference)
"""Causal self-attention (B=2, T=2048, C=1024, H=16, RoPE) on 8 TRN2 NeuronCores.

Sharding: core i handles batch b = i//4 and head group g = i%4 (4 heads each).
Each core computes q/k (transposed, RoPE'd), v, causal attention, and a partial
output projection; the host sums the 4 partials per batch element (tensor-
parallel unshard) and adds the constant term b_proj + b_v @ W_proj, which is
independent of x because softmax rows sum to 1.

Layout strategy (no on-chip transposes):
  - host passes x^T  [C, T]
  - q^T, k^T computed as (W^T x^T) with j (head*dim) on partitions
  - rotate_half(q) computed on-chip as P @ q^T (signed permutation matmul)
  - v computed in natural [t, j] layout, augmented with a ones column so the
    attention-value matmul also produces the softmax denominator
  - scores computed transposed: s^T[k, q] = k^T(d,k)^T . q^T(d,q); softmax
    normalization deferred until after AV (flash-style), no max subtraction
    (scores are ~N(0,1); exp is safe in fp32)
  - output projection consumes y^T directly: out[t, c] = y^T(j,t)^T . Wp(j,c)
All matmuls use the float32r dtype view (fp32 bits, ~4x faster PE path).
"""

import numpy as np

B, T, C, H, D = 2, 2048, 1024, 16, 64
G = 4           # heads per core
NCORES = 8
TS = 512        # t / q super-tile width
NT = T // 128   # 16 t-blocks
NTS = T // TS   # 4 t-supers
MASK_VAL = -1e5

_cached = {}


def _apply_workarounds():
    """This neuronxcc build rejects TPB instructions with >1 embedded sem wait.
    Patch the Tile drain and add a BIR pass splitting extra waits into
    standalone EventSemaphore instructions on the same (in-order) engine."""
    import concourse.tile as tile
    import concourse.mybir as mybir
    from concourse.vector_clock import ScopedClock

    if getattr(tile.TileContext, "_multiwait_patched", False):
        return

    def _drain_and_barrier(self, tick_clock, wait_clock):
        nc = self.nc
        probe = nc.sync.nop(nofuse=True)
        wait_clock.add_sem_waits(probe.ins, ScopedClock({None: tick_clock.global_clock}))
        si = probe.ins.sync_info
        waits = list(si.on_wait) if si and si.on_wait else []
        if si is not None:
            si.on_wait = []
        by_num = {h.num: h for h in self.sems.allocated().values()}
        for w in waits:
            nc.sync.wait_ge(by_num[w.id], w.wait_value)
        nc.sync.drain()
        nc.all_engine_barrier()
        popped = nc._tile_sem_poison_stack.pop()
        assert popped is self._sem_poison
        nc.clear_and_free_semaphores(list(self.sems.allocated().values()))
        nc.all_engine_barrier()

    tile.TileContext._drain_and_barrier = _drain_and_barrier
    tile.TileContext._multiwait_patched = True


def _split_multiwaits(nc, maxw=1):
    import concourse.mybir as mybir

    n = 0
    for f in nc.m.functions:
        for bb in f.blocks:
            insts = list(bb.instructions)
            out = []
            changed = False
            for inst in insts:
                si = inst.sync_info
                waits = list(si.on_wait) if si and si.on_wait else []
                if len(waits) > maxw:
                    for k, w in enumerate(waits[: len(waits) - maxw]):
                        out.append(
                            mybir.InstEventSemaphore(
                                name=f"{inst.name}-xw{k}",
                                engine=inst.engine,
                                ins=[],
                                outs=[],
                                sync_info=mybir.SyncInfo(on_wait=[w], on_update=[]),
                            )
                        )
                        n += 1
                    si.on_wait = waits[len(waits) - maxw :]
                    changed = True
                out.append(inst)
            if changed:
                bb.instructions.clear()
                for i in out:
                    bb.add_instruction(i)
    return n


def _build():
    import concourse.bass as bass
    import concourse.mybir as mybir
    import concourse.tile as tile

    _apply_workarounds()

    f32 = mybir.dt.float32
    f32r = mybir.dt.float32r
    Exp = mybir.ActivationFunctionType.Exp
    Ident = mybir.ActivationFunctionType.Identity

    def r(ap):
        return ap.bitcast(f32r)

    def g(ap):
        return ap  # f32r handled natively by DVE/ACT

    nc = bass.Bass()

    xT = nc.dram_tensor("xT", [C, T], f32r, kind="ExternalInput")
    w1 = nc.dram_tensor("w1", [C, 512], f32r, kind="ExternalInput")     # [q01 q23 k01 k23]
    b1 = nc.dram_tensor("b1", [128, 4], f32, kind="ExternalInput")
    wv = nc.dram_tensor("wv", [C, 256], f32r, kind="ExternalInput")
    wp = nc.dram_tensor("wp", [256, C], f32r, kind="ExternalInput")
    cosb = nc.dram_tensor("cosb", [128, T], f32, kind="ExternalInput")
    sinb = nc.dram_tensor("sinb", [128, T], f32, kind="ExternalInput")
    masks = nc.dram_tensor("masks", [128, 4 * TS], f32, kind="ExternalInput")
    pt2 = nc.dram_tensor("pt2", [128, 128], f32r, kind="ExternalInput")  # rotate-half perm^T
    out = nc.dram_tensor("out", [T, C], f32, kind="ExternalOutput")
    scr = nc.dram_tensor("scr", [16, TS], f32)                          # recip bounce

    with tile.TileContext(nc) as tc:
        with (
            tc.tile_pool(name="persist", bufs=1) as per,
            tc.tile_pool(name="xq", bufs=9) as xq,
            tc.tile_pool(name="qkr", bufs=3) as qkrp,
            tc.tile_pool(name="tmp", bufs=3) as tmp,
            tc.tile_pool(name="pp", bufs=3) as pp,
            tc.tile_pool(name="rr", bufs=2) as rr,
            tc.tile_pool(name="yo", bufs=2) as yo,
            tc.tile_pool(name="psA", bufs=3, space="PSUM") as psA,
            tc.tile_pool(name="psO", bufs=2, space="PSUM") as psO,
        ):
            # ---- persistent tiles ----
            w1_s = per.tile([128, 8, 512], f32r)
            wv_s = per.tile([128, 8, 256], f32r)
            wp_s = per.tile([128, 2, C], f32r)
            b1_s = per.tile([128, 4], f32)
            cos_s = per.tile([128, T], f32)
            sin_s = per.tile([128, T], f32)
            msk_s = per.tile([128, 4 * TS], f32)
            pt2_s = per.tile([128, 128], f32r)
            qk_s = per.tile([128, 4, T], f32r)        # [q01' q23' k01' k23']
            # v storage per head pair:
            #   [v_even(0:64) | ones(64:66) | gap(66:97) | v_odd(97:161)]
            # The AV lhsT is a 128-wide window: even head -> cols 0..127, so y
            # lands in psum rows 0..63 with the denominator in row 64; odd
            # head -> cols 33..160, so y lands in rows 64..127 with the
            # denominator (ones col 65) in row 32. Engine ops require
            # 32-aligned partition starts, so denominator rows must be 32/64.
            v_s = per.tile([128, NT, 2, 161], f32r)
            yT_s = per.tile([128, 2, T], f32r)

            for cb in range(8):
                nc.sync.dma_start(out=w1_s[:, cb, :], in_=w1[cb * 128:(cb + 1) * 128, :])
                nc.sync.dma_start(out=wv_s[:, cb, :], in_=wv[cb * 128:(cb + 1) * 128, :])
            for jb in range(2):
                nc.sync.dma_start(out=wp_s[:, jb, :], in_=wp[jb * 128:(jb + 1) * 128, :])
            nc.sync.dma_start(out=b1_s, in_=b1[:])
            nc.sync.dma_start(out=cos_s, in_=cosb[:])
            nc.sync.dma_start(out=sin_s, in_=sinb[:])
            nc.sync.dma_start(out=msk_s, in_=masks[:])
            nc.sync.dma_start(out=pt2_s, in_=pt2[:])
            nc.vector.memset(v_s.bitcast(f32).rearrange("p a b c -> p (a b c)"), 1.0)

            # ---- phase 1: q/k projection + RoPE, v projection ----
            for ts in range(NTS):
                tsl = slice(ts * TS, (ts + 1) * TS)
                xts = []
                for cb in range(8):
                    xt = xq.tile([128, TS], f32r, tag="xts")
                    nc.sync.dma_start(out=xt, in_=xT[cb * 128:(cb + 1) * 128, tsl])
                    xts.append(xt)
                for jb in range(4):
                    ps = psA.tile([128, 512], f32, tag="mm")
                    for cb in range(8):
                        nc.tensor.matmul(
                            ps,
                            r(w1_s[:, cb, jb * 128:(jb + 1) * 128]),
                            r(xts[cb]),
                            start=(cb == 0),
                            stop=(cb == 7),
                        )
                    qkr = qkrp.tile([128, TS], f32r, tag="qkr")
                    nc.scalar.activation(g(qkr), ps, Ident, bias=b1_s[:, jb:jb + 1], scale=1.0)
                    psr = psA.tile([128, 512], f32, tag="mm")
                    nc.tensor.matmul(psr, r(pt2_s), r(qkr), start=True, stop=True)
                    t1 = tmp.tile([128, TS], f32, tag="t1")
                    nc.vector.tensor_mul(t1, g(qkr), cos_s[:, tsl])
                    t2 = tmp.tile([128, TS], f32, tag="t2")
                    nc.vector.tensor_mul(t2, psr, sin_s[:, tsl])
                    nc.vector.tensor_add(g(qk_s)[:, jb, tsl], t1, t2)
                for tb2 in range(4):
                    tb = ts * 4 + tb2
                    psv = psA.tile([128, 512], f32, tag="mm")
                    for cb in range(8):
                        nc.tensor.matmul(
                            psv[:, :256],
                            r(xts[cb][:, tb2 * 128:(tb2 + 1) * 128]),
                            r(wv_s[:, cb, :]),
                            start=(cb == 0),
                            stop=(cb == 7),
                        )
                    psv4 = psv[:, :256].rearrange("p (pr par d) -> p pr par d", par=2, d=D)
                    nc.vector.tensor_copy(g(v_s)[:, tb, :, 0:64], psv4[:, :, 0, :])
                    nc.vector.tensor_copy(g(v_s)[:, tb, :, 97:161], psv4[:, :, 1, :])

            # ---- phase 2: attention (software-pipelined so PE never waits) ----
            for h in range(G):
                par = h % 2
                prow = slice(par * 64, par * 64 + 64)
                srow = 64 - 32 * par  # denominator row in the AV psum (32-aligned)
                qT = qk_s[prow, h // 2, :]
                kT = qk_s[prow, 2 + h // 2, :]

                def v_win(kb, pair=h // 2, par=par):
                    # 128-wide lhsT window into the [v_even |1|1| v_odd] slot
                    w = v_s[:, kb, pair, :]
                    return w[:, 33 * par:33 * par + 128]

                for js in range(NTS):
                    qsl = slice(js * TS, (js + 1) * TS)
                    po = psO.tile([128, 512], f32, tag="av")
                    nkb = 4 * js + 4
                    pend = None
                    for kb in range(nkb):
                        pss = psA.tile([128, 512], f32, tag="mm")
                        nc.tensor.matmul(
                            pss,
                            r(kT[:, kb * 128:(kb + 1) * 128]),
                            r(qT[:, qsl]),
                            start=True,
                            stop=True,
                        )
                        roff = kb - 4 * js
                        if roff >= 0:
                            sm = tmp.tile([128, TS], f32, tag="sm")
                            nc.vector.tensor_add(
                                sm, pss, msk_s[:, roff * TS:(roff + 1) * TS]
                            )
                            src = sm
                        else:
                            src = pss
                        pt = pp.tile([128, TS], f32r, tag="pt")
                        nc.scalar.activation(g(pt), src, Exp, scale=0.125)
                        if pend is not None:
                            nc.tensor.matmul(po, r(v_win(kb - 1)), r(pend),
                                             start=(kb == 1), stop=False)
                        pend = pt
                    nc.tensor.matmul(po, r(v_win(nkb - 1)), r(pend),
                                     start=False, stop=True)
                    # normalize: 1/S broadcast to 64 partitions via DRAM bounce
                    rs = rr.tile([65, TS], f32, tag="rs")
                    nc.vector.reciprocal(rs[srow:srow + 1, :], po[srow:srow + 1, :])
                    slot = h * 4 + js
                    nc.sync.dma_start(out=scr[slot:slot + 1, :], in_=rs[srow:srow + 1, :])
                    rb = rr.tile([128, TS], f32, tag="rb")
                    sc = scr[slot:slot + 1, :]
                    nc.gpsimd.dma_start(
                        out=rb[prow, :],
                        in_=bass.AP(tensor=sc.tensor, offset=sc.offset,
                                    ap=[[0, 64]] + list(sc.ap[1:])),
                    )
                    nc.vector.tensor_mul(g(yT_s)[prow, h // 2, qsl], po[prow, :], rb[prow, :])

            # ---- phase 3: output projection ----
            for tb in range(NT):
                for cs in range(2):
                    py = psA.tile([128, 512], f32, tag="mm")
                    for jb in range(2):
                        nc.tensor.matmul(
                            py,
                            r(yT_s[:, jb, tb * 128:(tb + 1) * 128]),
                            r(wp_s[:, jb, cs * 512:(cs + 1) * 512]),
                            start=(jb == 0),
                            stop=(jb == 1),
                        )
                    ot = yo.tile([128, 512], f32, tag="ot")
                    nc.vector.tensor_copy(ot, py)
                    nc.sync.dma_start(
                        out=out[tb * 128:(tb + 1) * 128, cs * 512:(cs + 1) * 512],
                        in_=ot,
                    )

    _split_multiwaits(nc)
    return nc


def _rot_cols(w):
    """rotate_half as a column transform: out[:, d] = -w[:, d+32] (d<32), w[:, d-32] (d>=32)."""
    o = np.empty_like(w)
    o[..., :32] = -w[..., 32:64]
    o[..., 32:] = w[..., :32]
    return o


def _host_inputs(x, W_attn, b_attn, W_proj):
    f32 = np.float32
    inv = (1.0 / (10000.0 ** (np.arange(0, D, 2, dtype=f32) / f32(D)))).astype(f32)
    t = np.arange(T, dtype=f32)
    ang = np.outer(inv, t).astype(f32)            # [32, T]
    cos32, sin32 = np.cos(ang).astype(f32), np.sin(ang).astype(f32)
    cosb = np.tile(cos32, (4, 1))                  # [128, T], row p -> freq p%32
    sinb = np.tile(sin32, (4, 1))

    kk = np.arange(128)[:, None]
    qq = np.arange(TS)[None, :]
    masks = np.concatenate(
        [np.where(qq >= kk + 128 * rr_, f32(0), f32(MASK_VAL)) for rr_ in range(4)],
        axis=1,
    ).astype(f32)                                  # [128, 4*TS]

    p64 = np.zeros((D, D), dtype=f32)
    for d in range(32):
        p64[d, d + 32] = -1.0
        p64[d + 32, d] = 1.0
    pt2 = np.zeros((128, 128), dtype=f32)
    pt2[:64, :64] = p64.T
    pt2[64:, 64:] = p64.T

    xTs = [np.ascontiguousarray(x[b].T, dtype=f32) for b in range(B)]

    per_g = []
    for g in range(G):
        hs = [4 * g + j for j in range(G)]
        qcols = [W_attn[:, h * D:(h + 1) * D] for h in hs]
        kcols = [W_attn[:, C + h * D:C + (h + 1) * D] for h in hs]
        qb = [b_attn[h * D:(h + 1) * D] for h in hs]
        kb_ = [b_attn[C + h * D:C + (h + 1) * D] for h in hs]
        w1 = np.concatenate(
            [qcols[0], qcols[1], qcols[2], qcols[3], kcols[0], kcols[1], kcols[2], kcols[3]],
            axis=1,
        ).astype(f32)                              # [C, 512]: [q01 q23 k01 k23]
        b1 = np.concatenate(qb + kb_).astype(f32).reshape(4, 128).T.copy()  # [128, 4]
        wv_ = W_attn[:, 2 * C + 256 * g:2 * C + 256 * (g + 1)].astype(f32)
        wp_ = W_proj[256 * g:256 * (g + 1), :].astype(f32)
        per_g.append((w1, b1, wv_, wp_))

    shared = dict(cosb=cosb, sinb=sinb, masks=masks, pt2=pt2)
    in_maps = []
    for i in range(NCORES):
        b, g = i // 4, i % 4
        w1, b1, wv_, wp_ = per_g[g]
        in_maps.append(dict(xT=xTs[b], w1=w1, b1=b1, wv=wv_, wp=wp_, **shared))
    return in_maps


def kernel(x, W_attn, b_attn, W_proj, b_proj):
    from concourse.bass_utils import run_bass_kernel_spmd

    x = np.asarray(x, dtype=np.float32)
    W_attn = np.asarray(W_attn, dtype=np.float32)
    b_attn = np.asarray(b_attn, dtype=np.float32)
    W_proj = np.asarray(W_proj, dtype=np.float32)
    b_proj = np.asarray(b_proj, dtype=np.float32)

    if "nc" not in _cached:
        _cached["nc"] = _build()
    nc = _cached["nc"]

    in_maps = _host_inputs(x, W_attn, b_attn, W_proj)
    res = run_bass_kernel_spmd(nc, in_maps, core_ids=list(range(NCORES)))
    _cached["last_results"] = res

    const = (b_proj + b_attn[2 * C:] @ W_proj).astype(np.float32)
    y = np.empty((B, T, C), dtype=np.float32)
    for b in range(B):
        acc = res.results[4 * b]["out"].astype(np.float32).copy()
        for g in range(1, 4):
            acc += res.results[4 * b + g]["out"]
        y[b] = acc + const
    return y


# revision 21
# speedup vs baseline: 1.4103x; 1.4103x over previous
"""Causal self-attention (B=2, T=2048, C=1024, H=16, RoPE) on 8 TRN2 NeuronCores.

Sharding: core i handles batch b = i//4 and head group g = i%4 (4 heads each).
Each core computes q/k (transposed, RoPE'd), v, causal attention, and a partial
output projection; the host sums the 4 partials per batch element (tensor-
parallel unshard) and adds the constant term b_proj + b_v @ W_proj, which is
independent of x because softmax rows sum to 1.

Layout strategy (no on-chip transposes):
  - host passes x^T  [C, T]
  - q^T, k^T computed as (W^T x^T) with j (head*dim) on partitions
  - rotate_half(q) computed on-chip as P @ q^T (signed permutation matmul)
  - v computed in natural [t, j] layout, augmented with a ones column so the
    attention-value matmul also produces the softmax denominator
  - scores computed transposed: s^T[k, q] = k^T(d,k)^T . q^T(d,q); softmax
    normalization deferred until after AV (flash-style), no max subtraction
    (scores are ~N(0,1); exp is safe in fp32)
  - output projection consumes y^T directly: out[t, c] = y^T(j,t)^T . Wp(j,c)
Matmul operands are bf16 (full PE rate incl. 512-wide moving operands --
measured f32r falls to half rate there); softmax/RoPE arithmetic stays f32.
"""

import numpy as np

B, T, C, H, D = 2, 2048, 1024, 16, 64
G = 4           # heads per core
NCORES = 8
TS = 512        # t / q super-tile width
NT = T // 128   # 16 t-blocks
NTS = T // TS   # 4 t-supers
MASK_VAL = -1e5

_cached = {}


def _apply_workarounds():
    """This neuronxcc build rejects TPB instructions with >1 embedded sem wait.
    Patch the Tile drain and add a BIR pass splitting extra waits into
    standalone EventSemaphore instructions on the same (in-order) engine."""
    import concourse.tile as tile
    import concourse.mybir as mybir
    from concourse.vector_clock import ScopedClock

    if getattr(tile.TileContext, "_multiwait_patched", False):
        return

    def _drain_and_barrier(self, tick_clock, wait_clock):
        nc = self.nc
        probe = nc.sync.nop(nofuse=True)
        wait_clock.add_sem_waits(probe.ins, ScopedClock({None: tick_clock.global_clock}))
        si = probe.ins.sync_info
        waits = list(si.on_wait) if si and si.on_wait else []
        if si is not None:
            si.on_wait = []
        by_num = {h.num: h for h in self.sems.allocated().values()}
        for w in waits:
            nc.sync.wait_ge(by_num[w.id], w.wait_value)
        nc.sync.drain()
        nc.all_engine_barrier()
        popped = nc._tile_sem_poison_stack.pop()
        assert popped is self._sem_poison
        nc.clear_and_free_semaphores(list(self.sems.allocated().values()))
        nc.all_engine_barrier()

    tile.TileContext._drain_and_barrier = _drain_and_barrier
    tile.TileContext._multiwait_patched = True


def _split_multiwaits(nc, maxw=1):
    import concourse.mybir as mybir

    n = 0
    for f in nc.m.functions:
        for bb in f.blocks:
            insts = list(bb.instructions)
            out = []
            changed = False
            for inst in insts:
                si = inst.sync_info
                waits = list(si.on_wait) if si and si.on_wait else []
                if len(waits) > maxw:
                    for k, w in enumerate(waits[: len(waits) - maxw]):
                        out.append(
                            mybir.InstEventSemaphore(
                                name=f"{inst.name}-xw{k}",
                                engine=inst.engine,
                                ins=[],
                                outs=[],
                                sync_info=mybir.SyncInfo(on_wait=[w], on_update=[]),
                            )
                        )
                        n += 1
                    si.on_wait = waits[len(waits) - maxw :]
                    changed = True
                out.append(inst)
            if changed:
                bb.instructions.clear()
                for i in out:
                    bb.add_instruction(i)
    return n


def _build():
    import concourse.bass as bass
    import concourse.mybir as mybir
    import concourse.tile as tile

    _apply_workarounds()

    f32 = mybir.dt.float32
    bf16 = mybir.dt.bfloat16
    Exp = mybir.ActivationFunctionType.Exp
    Ident = mybir.ActivationFunctionType.Identity


    nc = bass.Bass()

    xT = nc.dram_tensor("xT", [C, T], bf16, kind="ExternalInput")
    w1 = nc.dram_tensor("w1", [C, 512], bf16, kind="ExternalInput")     # [q01 q23 k01 k23]
    b1 = nc.dram_tensor("b1", [128, 4], f32, kind="ExternalInput")
    wv = nc.dram_tensor("wv", [C, 256], bf16, kind="ExternalInput")
    wp = nc.dram_tensor("wp", [256, C], bf16, kind="ExternalInput")
    cosb = nc.dram_tensor("cosb", [128, T], f32, kind="ExternalInput")
    sinb = nc.dram_tensor("sinb", [128, T], f32, kind="ExternalInput")
    masks = nc.dram_tensor("masks", [128, 4 * TS], f32, kind="ExternalInput")
    pt2 = nc.dram_tensor("pt2", [128, 128], bf16, kind="ExternalInput")  # rotate-half perm^T
    out = nc.dram_tensor("out", [T, C], f32, kind="ExternalOutput")
    scr = nc.dram_tensor("scr", [16, TS], f32)                          # S bounce
    scr2 = nc.dram_tensor("scr2", [16, TS], f32)                        # 1/S bounce

    with tile.TileContext(nc) as tc:
        with (
            tc.tile_pool(name="persist", bufs=1) as per,
            tc.tile_pool(name="xq", bufs=12) as xq,
            tc.tile_pool(name="qkr", bufs=4) as qkrp,
            tc.tile_pool(name="tmp", bufs=4) as tmp,
            tc.tile_pool(name="pp", bufs=6) as pp,
            tc.tile_pool(name="rr", bufs=4) as rr,
            tc.tile_pool(name="yo", bufs=4) as yo,
            tc.tile_pool(name="psA", bufs=4, space="PSUM") as psA,
            tc.tile_pool(name="psO", bufs=2, space="PSUM") as psO,
        ):
            # ---- persistent tiles ----
            w1_s = per.tile([128, 8, 512], bf16)
            wv_s = per.tile([128, 8, 256], bf16)
            wp_s = per.tile([128, 2, C], bf16)
            b1_s = per.tile([128, 4], f32)
            cos_s = per.tile([128, T], f32)
            sin_s = per.tile([128, T], f32)
            msk_s = per.tile([128, 4 * TS], f32)
            pt2_s = per.tile([128, 128], bf16)
            qk_s = per.tile([128, 4, T], bf16)        # [q01' q23' k01' k23']
            # v storage per head pair:
            #   [v_even(0:64) | ones(64:66) | gap(66:97) | v_odd(97:161)]
            # The AV lhsT is a 128-wide window: even head -> cols 0..127, so y
            # lands in psum rows 0..63 with the denominator in row 64; odd
            # head -> cols 33..160, so y lands in rows 64..127 with the
            # denominator (ones col 65) in row 32. Engine ops require
            # 32-aligned partition starts, so denominator rows must be 32/64.
            v_s = per.tile([128, NT, 2, 161], bf16)
            yT_s = per.tile([128, 2, T], bf16)

            for cb in range(8):
                nc.sync.dma_start(out=w1_s[:, cb, :], in_=w1[cb * 128:(cb + 1) * 128, :])
                nc.sync.dma_start(out=wv_s[:, cb, :], in_=wv[cb * 128:(cb + 1) * 128, :])
            for jb in range(2):
                nc.sync.dma_start(out=wp_s[:, jb, :], in_=wp[jb * 128:(jb + 1) * 128, :])
            nc.sync.dma_start(out=b1_s, in_=b1[:])
            nc.sync.dma_start(out=cos_s, in_=cosb[:])
            nc.sync.dma_start(out=sin_s, in_=sinb[:])
            nc.sync.dma_start(out=msk_s, in_=masks[:])
            nc.sync.dma_start(out=pt2_s, in_=pt2[:])
            nc.vector.memset(v_s.rearrange("p a b c -> p (a b c)"), 1.0)

            # ---- phase 1: q/k projection + RoPE, v projection ----
            for ts in range(NTS):
                tsl = slice(ts * TS, (ts + 1) * TS)
                xts = []
                for cb in range(8):
                    xt = xq.tile([128, TS], bf16, tag="xts")
                    nc.sync.dma_start(out=xt, in_=xT[cb * 128:(cb + 1) * 128, tsl])
                    xts.append(xt)
                for jb in range(4):
                    ps = psA.tile([128, 512], f32, tag="mm")
                    for cb in range(8):
                        nc.tensor.matmul(
                            ps,
                            w1_s[:, cb, jb * 128:(jb + 1) * 128],
                            xts[cb],
                            start=(cb == 0),
                            stop=(cb == 7),
                        )
                    qkr = qkrp.tile([128, TS], bf16, tag="qkr")
                    nc.scalar.activation(qkr, ps, Ident, bias=b1_s[:, jb:jb + 1], scale=1.0)
                    psr = psA.tile([128, 512], f32, tag="mm")
                    nc.tensor.matmul(psr, pt2_s, qkr, start=True, stop=True)
                    t1 = tmp.tile([128, TS], f32, tag="t1")
                    nc.vector.tensor_mul(t1, qkr, cos_s[:, tsl])
                    t2 = tmp.tile([128, TS], f32, tag="t2")
                    nc.vector.tensor_mul(t2, psr, sin_s[:, tsl])
                    nc.vector.tensor_add(qk_s[:, jb, tsl], t1, t2)
                for tb2 in range(4):
                    tb = ts * 4 + tb2
                    psv = psA.tile([128, 512], f32, tag="mm")
                    for cb in range(8):
                        nc.tensor.matmul(
                            psv[:, :256],
                            xts[cb][:, tb2 * 128:(tb2 + 1) * 128],
                            wv_s[:, cb, :],
                            start=(cb == 0),
                            stop=(cb == 7),
                        )
                    psv4 = psv[:, :256].rearrange("p (pr par d) -> p pr par d", par=2, d=D)
                    nc.vector.tensor_copy(v_s[:, tb, :, 0:64], psv4[:, :, 0, :])
                    nc.vector.tensor_copy(v_s[:, tb, :, 97:161], psv4[:, :, 1, :])

            # ---- phase 2+3: attention, with output projection interleaved
            # per q-super so PE fills ACT-bound stretches with oproj matmuls.
            for js in range(NTS):
                qsl = slice(js * TS, (js + 1) * TS)
                nkb = 4 * js + 4
                for h in range(G):
                    par = h % 2
                    prow = slice(par * 64, par * 64 + 64)
                    srow = 64 - 32 * par  # denominator row (32-aligned)
                    qT = qk_s[prow, h // 2, :]
                    kT = qk_s[prow, 2 + h // 2, :]

                    def v_win(kb, pair=h // 2, par=par):
                        # 128-wide lhsT window into the [v_even |1|1| v_odd] slot
                        return v_s[:, kb, pair, 33 * par:33 * par + 128]

                    po = psO.tile([128, 512], f32, tag="av")
                    pend = None
                    for kb in range(nkb):
                        pss = psA.tile([128, 512], f32, tag="mm")
                        nc.tensor.matmul(
                            pss,
                            kT[:, kb * 128:(kb + 1) * 128],
                            qT[:, qsl],
                            start=True,
                            stop=True,
                        )
                        pt = pp.tile([128, TS], bf16, tag="pt")
                        roff = kb - 4 * js
                        if roff >= 0:
                            # columns qq < 128*roff are fully masked: skip the
                            # mask-add and exp there, just zero them.
                            w0 = 128 * roff
                            sm = tmp.tile([128, TS], f32, tag="sm")
                            nc.vector.tensor_add(
                                sm[:, w0:], pss[:, w0:],
                                msk_s[:, roff * TS + w0:(roff + 1) * TS],
                            )
                            if w0:
                                nc.vector.memset(pt[:, :w0], 0.0)
                            nc.scalar.activation(pt[:, w0:], sm[:, w0:], Exp, scale=0.125)
                        else:
                            nc.scalar.activation(pt, pss, Exp, scale=0.125)
                        if pend is not None:
                            nc.tensor.matmul(po, v_win(kb - 1), pend,
                                             start=(kb == 1), stop=False)
                        pend = pt
                    nc.tensor.matmul(po, v_win(nkb - 1), pend,
                                     start=False, stop=True)
                    # normalize y = po * (1/S). DVE InstReciprocal costs ~6.3
                    # cyc/elem along the free dim, so spread S across 64
                    # partitions (DRAM bounce) before taking the reciprocal,
                    # then bounce back as a partition-broadcast row.
                    slot = h * 4 + js
                    rs = rr.tile([65, TS], f32, tag="rs")
                    nc.vector.tensor_copy(rs[srow:srow + 1, :], po[srow:srow + 1, :])
                    nc.sync.dma_start(out=scr[slot:slot + 1, :], in_=rs[srow:srow + 1, :])
                    rv = rr.tile([64, 8], f32, tag="rv")
                    nc.sync.dma_start(
                        out=rv,
                        in_=scr[slot:slot + 1, :].rearrange("a (p f) -> (a p) f", p=64),
                    )
                    rvr = rr.tile([64, 8], f32, tag="rvr")
                    nc.vector.reciprocal(rvr, rv)
                    nc.sync.dma_start(
                        out=scr2[slot:slot + 1, :].rearrange("a (p f) -> (a p) f", p=64),
                        in_=rvr,
                    )
                    rb = rr.tile([128, TS], f32, tag="rb")
                    sc = scr2[slot:slot + 1, :]
                    nc.gpsimd.dma_start(
                        out=rb[prow, :],
                        in_=bass.AP(tensor=sc.tensor, offset=sc.offset,
                                    ap=[[0, 64]] + list(sc.ap[1:])),
                    )
                    nc.vector.tensor_mul(yT_s[prow, h // 2, qsl], po[prow, :], rb[prow, :])

                # output projection for the t-blocks this q-super completed
                for tb in range(4 * js, 4 * js + 4):
                    for cs in range(2):
                        py = psA.tile([128, 512], f32, tag="mm")
                        for jb in range(2):
                            nc.tensor.matmul(
                                py,
                                yT_s[:, jb, tb * 128:(tb + 1) * 128],
                                wp_s[:, jb, cs * 512:(cs + 1) * 512],
                                start=(jb == 0),
                                stop=(jb == 1),
                            )
                        ot = yo.tile([128, 512], f32, tag="ot")
                        nc.vector.tensor_copy(ot, py)
                        nc.sync.dma_start(
                            out=out[tb * 128:(tb + 1) * 128, cs * 512:(cs + 1) * 512],
                            in_=ot,
                        )

    _split_multiwaits(nc)
    return nc


def _rot_cols(w):
    """rotate_half as a column transform: out[:, d] = -w[:, d+32] (d<32), w[:, d-32] (d>=32)."""
    o = np.empty_like(w)
    o[..., :32] = -w[..., 32:64]
    o[..., 32:] = w[..., :32]
    return o


def _host_inputs(x, W_attn, b_attn, W_proj):
    f32 = np.float32
    inv = (1.0 / (10000.0 ** (np.arange(0, D, 2, dtype=f32) / f32(D)))).astype(f32)
    t = np.arange(T, dtype=f32)
    ang = np.outer(inv, t).astype(f32)            # [32, T]
    cos32, sin32 = np.cos(ang).astype(f32), np.sin(ang).astype(f32)
    cosb = np.tile(cos32, (4, 1))                  # [128, T], row p -> freq p%32
    sinb = np.tile(sin32, (4, 1))

    kk = np.arange(128)[:, None]
    qq = np.arange(TS)[None, :]
    masks = np.concatenate(
        [np.where(qq >= kk + 128 * rr_, f32(0), f32(MASK_VAL)) for rr_ in range(4)],
        axis=1,
    ).astype(f32)                                  # [128, 4*TS]

    import ml_dtypes

    bf16 = ml_dtypes.bfloat16
    p64 = np.zeros((D, D), dtype=f32)
    for d in range(32):
        p64[d, d + 32] = -1.0
        p64[d + 32, d] = 1.0
    pt2 = np.zeros((128, 128), dtype=f32)
    pt2[:64, :64] = p64.T
    pt2[64:, 64:] = p64.T
    pt2 = pt2.astype(bf16)

    xTs = [np.ascontiguousarray(x[b].T).astype(bf16) for b in range(B)]

    per_g = []
    for g in range(G):
        hs = [4 * g + j for j in range(G)]
        qcols = [W_attn[:, h * D:(h + 1) * D] for h in hs]
        kcols = [W_attn[:, C + h * D:C + (h + 1) * D] for h in hs]
        qb = [b_attn[h * D:(h + 1) * D] for h in hs]
        kb_ = [b_attn[C + h * D:C + (h + 1) * D] for h in hs]
        w1 = np.concatenate(
            [qcols[0], qcols[1], qcols[2], qcols[3], kcols[0], kcols[1], kcols[2], kcols[3]],
            axis=1,
        ).astype(bf16)                             # [C, 512]: [q01 q23 k01 k23]
        b1 = np.concatenate(qb + kb_).astype(f32).reshape(4, 128).T.copy()  # [128, 4]
        wv_ = W_attn[:, 2 * C + 256 * g:2 * C + 256 * (g + 1)].astype(bf16)
        wp_ = W_proj[256 * g:256 * (g + 1), :].astype(bf16)
        per_g.append((w1, b1, wv_, wp_))

    shared = dict(cosb=cosb, sinb=sinb, masks=masks, pt2=pt2)
    in_maps = []
    for i in range(NCORES):
        b, g = i // 4, i % 4
        w1, b1, wv_, wp_ = per_g[g]
        in_maps.append(dict(xT=xTs[b], w1=w1, b1=b1, wv=wv_, wp=wp_, **shared))
    return in_maps


def kernel(x, W_attn, b_attn, W_proj, b_proj):
    from concourse.bass_utils import run_bass_kernel_spmd

    x = np.asarray(x, dtype=np.float32)
    W_attn = np.asarray(W_attn, dtype=np.float32)
    b_attn = np.asarray(b_attn, dtype=np.float32)
    W_proj = np.asarray(W_proj, dtype=np.float32)
    b_proj = np.asarray(b_proj, dtype=np.float32)

    if "nc" not in _cached:
        _cached["nc"] = _build()
    nc = _cached["nc"]

    in_maps = _host_inputs(x, W_attn, b_attn, W_proj)
    res = run_bass_kernel_spmd(nc, in_maps, core_ids=list(range(NCORES)))
    _cached["last_results"] = res

    const = (b_proj + b_attn[2 * C:] @ W_proj).astype(np.float32)
    y = np.empty((B, T, C), dtype=np.float32)
    for b in range(B):
        acc = res.results[4 * b]["out"].astype(np.float32).copy()
        for g in range(1, 4):
            acc += res.results[4 * b + g]["out"]
        y[b] = acc + const
    return y


# revision 26
# speedup vs baseline: 1.7139x; 1.2153x over previous
"""Causal self-attention (B=2, T=2048, C=1024, H=16, RoPE) on 8 TRN2 NeuronCores.

Sharding: core i handles batch b = i//4 and head group g = i%4 (4 heads each).
Each core computes q/k (transposed, RoPE'd), v, causal attention, and a partial
output projection; the host sums the 4 partials per batch element (tensor-
parallel unshard) and adds the constant term b_proj + b_v @ W_proj, which is
independent of x because softmax rows sum to 1.

Layout strategy (no on-chip transposes):
  - host passes x^T  [C, T]
  - q^T, k^T computed as (W^T x^T) with j (head*dim) on partitions
  - rotate_half(q) computed on-chip as P @ q^T (signed permutation matmul)
  - v computed in natural [t, j] layout, augmented with a ones column so the
    attention-value matmul also produces the softmax denominator
  - scores computed transposed: s^T[k, q] = k^T(d,k)^T . q^T(d,q); softmax
    normalization deferred until after AV (flash-style), no max subtraction
    (scores are ~N(0,1); exp is safe in fp32)
  - output projection consumes y^T directly: out[t, c] = y^T(j,t)^T . Wp(j,c)
Matmul operands are bf16 (full PE rate incl. 512-wide moving operands --
measured f32r falls to half rate there); softmax/RoPE arithmetic stays f32.
"""

import numpy as np

B, T, C, H, D = 2, 2048, 1024, 16, 64
G = 4           # heads per core
NCORES = 8
TS = 512        # t / q super-tile width
NT = T // 128   # 16 t-blocks
NTS = T // TS   # 4 t-supers
MASK_VAL = -1e5

_cached = {}


def _apply_workarounds():
    """This neuronxcc build rejects TPB instructions with >1 embedded sem wait.
    Patch the Tile drain and add a BIR pass splitting extra waits into
    standalone EventSemaphore instructions on the same (in-order) engine."""
    import concourse.tile as tile
    import concourse.mybir as mybir
    from concourse.vector_clock import ScopedClock

    if getattr(tile.TileContext, "_multiwait_patched", False):
        return

    def _drain_and_barrier(self, tick_clock, wait_clock):
        nc = self.nc
        probe = nc.sync.nop(nofuse=True)
        wait_clock.add_sem_waits(probe.ins, ScopedClock({None: tick_clock.global_clock}))
        si = probe.ins.sync_info
        waits = list(si.on_wait) if si and si.on_wait else []
        if si is not None:
            si.on_wait = []
        by_num = {h.num: h for h in self.sems.allocated().values()}
        for w in waits:
            nc.sync.wait_ge(by_num[w.id], w.wait_value)
        nc.sync.drain()
        nc.all_engine_barrier()
        popped = nc._tile_sem_poison_stack.pop()
        assert popped is self._sem_poison
        nc.clear_and_free_semaphores(list(self.sems.allocated().values()))
        nc.all_engine_barrier()

    tile.TileContext._drain_and_barrier = _drain_and_barrier
    tile.TileContext._multiwait_patched = True


def _split_multiwaits(nc, maxw=1):
    import concourse.mybir as mybir

    n = 0
    for f in nc.m.functions:
        for bb in f.blocks:
            insts = list(bb.instructions)
            out = []
            changed = False
            for inst in insts:
                si = inst.sync_info
                waits = list(si.on_wait) if si and si.on_wait else []
                if len(waits) > maxw:
                    for k, w in enumerate(waits[: len(waits) - maxw]):
                        out.append(
                            mybir.InstEventSemaphore(
                                name=f"{inst.name}-xw{k}",
                                engine=inst.engine,
                                ins=[],
                                outs=[],
                                sync_info=mybir.SyncInfo(on_wait=[w], on_update=[]),
                            )
                        )
                        n += 1
                    si.on_wait = waits[len(waits) - maxw :]
                    changed = True
                out.append(inst)
            if changed:
                bb.instructions.clear()
                for i in out:
                    bb.add_instruction(i)
    return n


def _build():
    import concourse.bass as bass
    import concourse.mybir as mybir
    import concourse.tile as tile

    _apply_workarounds()

    f32 = mybir.dt.float32
    bf16 = mybir.dt.bfloat16
    Exp = mybir.ActivationFunctionType.Exp
    Ident = mybir.ActivationFunctionType.Identity


    nc = bass.Bass()

    xT = nc.dram_tensor("xT", [C, T], bf16, kind="ExternalInput")
    w1 = nc.dram_tensor("w1", [C, 512], bf16, kind="ExternalInput")     # [q01 q23 k01 k23]
    b1 = nc.dram_tensor("b1", [128, 4], f32, kind="ExternalInput")
    wv = nc.dram_tensor("wv", [C, 256], bf16, kind="ExternalInput")
    wp = nc.dram_tensor("wp", [256, C], bf16, kind="ExternalInput")
    cosb = nc.dram_tensor("cosb", [128, T], f32, kind="ExternalInput")
    sinb = nc.dram_tensor("sinb", [128, T], f32, kind="ExternalInput")
    masks = nc.dram_tensor("masks", [128, 4 * TS], bf16, kind="ExternalInput")
    pt2 = nc.dram_tensor("pt2", [128, 128], bf16, kind="ExternalInput")  # rotate-half perm^T
    out = nc.dram_tensor("out", [T, C], f32, kind="ExternalOutput")
    scr = nc.dram_tensor("scr", [16, TS], f32)                          # S bounce
    scr2 = nc.dram_tensor("scr2", [16, TS], f32)                        # 1/S bounce

    with tile.TileContext(nc) as tc:
        with (
            tc.tile_pool(name="persist", bufs=1) as per,
            tc.tile_pool(name="xq", bufs=12) as xq,
            tc.tile_pool(name="qkr", bufs=4) as qkrp,
            tc.tile_pool(name="tmp", bufs=4) as tmp,
            tc.tile_pool(name="pp", bufs=6) as pp,
            tc.tile_pool(name="rr", bufs=4) as rr,
            tc.tile_pool(name="yo", bufs=4) as yo,
            tc.tile_pool(name="psA", bufs=4, space="PSUM") as psA,
            tc.tile_pool(name="psO", bufs=4, space="PSUM") as psO,
        ):
            # ---- persistent tiles ----
            w1_s = per.tile([128, 8, 512], bf16)
            wv_s = per.tile([128, 8, 256], bf16)
            wp_s = per.tile([128, 2, C], bf16)
            b1_s = per.tile([128, 4], f32)
            cos_s = per.tile([128, T], f32)
            sin_s = per.tile([128, T], f32)
            msk_s = per.tile([128, 4 * TS], bf16)
            pt2_s = per.tile([128, 128], bf16)
            qk_s = per.tile([128, 4, T], bf16)        # [q01' q23' k01' k23']
            # v storage per head pair:
            #   [v_even(0:64) | ones(64:66) | gap(66:97) | v_odd(97:161)]
            # The AV lhsT is a 128-wide window: even head -> cols 0..127, so y
            # lands in psum rows 0..63 with the denominator in row 64; odd
            # head -> cols 33..160, so y lands in rows 64..127 with the
            # denominator (ones col 65) in row 32. Engine ops require
            # 32-aligned partition starts, so denominator rows must be 32/64.
            v_s = per.tile([128, NT, 2, 161], bf16)
            yT_s = per.tile([128, 2, T], bf16)

            for cb in range(8):
                nc.sync.dma_start(out=w1_s[:, cb, :], in_=w1[cb * 128:(cb + 1) * 128, :])
                nc.sync.dma_start(out=wv_s[:, cb, :], in_=wv[cb * 128:(cb + 1) * 128, :])
            for jb in range(2):
                nc.sync.dma_start(out=wp_s[:, jb, :], in_=wp[jb * 128:(jb + 1) * 128, :])
            nc.sync.dma_start(out=b1_s, in_=b1[:])
            nc.sync.dma_start(out=cos_s, in_=cosb[:])
            nc.sync.dma_start(out=sin_s, in_=sinb[:])
            nc.sync.dma_start(out=msk_s, in_=masks[:])
            nc.sync.dma_start(out=pt2_s, in_=pt2[:])
            nc.vector.memset(v_s.rearrange("p a b c -> p (a b c)"), 1.0)

            # ---- phase 1: q/k projection + RoPE, v projection ----
            for ts in range(NTS):
                tsl = slice(ts * TS, (ts + 1) * TS)
                xts = []
                for cb in range(8):
                    xt = xq.tile([128, TS], bf16, tag="xts")
                    nc.sync.dma_start(out=xt, in_=xT[cb * 128:(cb + 1) * 128, tsl])
                    xts.append(xt)
                for jb in range(4):
                    ps = psA.tile([128, 512], f32, tag="mm")
                    for cb in range(8):
                        nc.tensor.matmul(
                            ps,
                            w1_s[:, cb, jb * 128:(jb + 1) * 128],
                            xts[cb],
                            start=(cb == 0),
                            stop=(cb == 7),
                        )
                    qkr = qkrp.tile([128, TS], bf16, tag="qkr")
                    nc.scalar.activation(qkr, ps, Ident, bias=b1_s[:, jb:jb + 1], scale=1.0)
                    psr = psA.tile([128, 512], f32, tag="mm")
                    nc.tensor.matmul(psr, pt2_s, qkr, start=True, stop=True)
                    t1 = tmp.tile([128, TS], f32, tag="t1")
                    nc.vector.tensor_mul(t1, qkr, cos_s[:, tsl])
                    t2 = tmp.tile([128, TS], f32, tag="t2")
                    nc.vector.tensor_mul(t2, psr, sin_s[:, tsl])
                    nc.vector.tensor_add(qk_s[:, jb, tsl], t1, t2)
                for tb2 in range(4):
                    tb = ts * 4 + tb2
                    psv = psA.tile([128, 512], f32, tag="mm")
                    for cb in range(8):
                        nc.tensor.matmul(
                            psv[:, :256],
                            xts[cb][:, tb2 * 128:(tb2 + 1) * 128],
                            wv_s[:, cb, :],
                            start=(cb == 0),
                            stop=(cb == 7),
                        )
                    psv4 = psv[:, :256].rearrange("p (pr par d) -> p pr par d", par=2, d=D)
                    nc.vector.tensor_copy(v_s[:, tb, :, 0:64], psv4[:, :, 0, :])
                    nc.vector.tensor_copy(v_s[:, tb, :, 97:161], psv4[:, :, 1, :])

            # ---- phase 2+3: attention, with output projection interleaved
            # per q-super so PE fills ACT-bound stretches with oproj matmuls.
            for js in range(NTS):
                qsl = slice(js * TS, (js + 1) * TS)
                nkb = 4 * js + 4
                for h in range(G):
                    par = h % 2
                    prow = slice(par * 64, par * 64 + 64)
                    srow = 64 - 32 * par  # denominator row (32-aligned)
                    qT = qk_s[prow, h // 2, :]
                    kT = qk_s[prow, 2 + h // 2, :]

                    def v_win(kb, pair=h // 2, par=par):
                        # 128-wide lhsT window into the [v_even |1|1| v_odd] slot
                        return v_s[:, kb, pair, 33 * par:33 * par + 128]

                    po = psO.tile([128, 512], f32, tag="av")
                    pend = None
                    for kb in range(nkb):
                        pss = psA.tile([128, 512], f32, tag="mm")
                        nc.tensor.matmul(
                            pss,
                            kT[:, kb * 128:(kb + 1) * 128],
                            qT[:, qsl],
                            start=True,
                            stop=True,
                        )
                        pt = pp.tile([128, TS], bf16, tag="pt")
                        roff = kb - 4 * js
                        if roff >= 0:
                            # columns qq < 128*roff are fully masked: skip exp
                            # there and zero them; the partially-masked rest is
                            # zeroed multiplicatively after exp (bf16 SBUF TT
                            # is cheaper than an f32 PSUM-operand mask add).
                            w0 = 128 * roff
                            if w0:
                                nc.vector.memset(pt[:, :w0], 0.0)
                            nc.scalar.activation(pt[:, w0:], pss[:, w0:], Exp, scale=0.125)
                            nc.vector.tensor_mul(
                                pt[:, w0:], pt[:, w0:],
                                msk_s[:, roff * TS + w0:(roff + 1) * TS],
                            )
                        else:
                            nc.scalar.activation(pt, pss, Exp, scale=0.125)
                        if pend is not None:
                            nc.tensor.matmul(po, v_win(kb - 1), pend,
                                             start=(kb == 1), stop=False)
                        pend = pt
                    nc.tensor.matmul(po, v_win(nkb - 1), pend,
                                     start=False, stop=True)
                    # normalize y = po * (1/S). DVE InstReciprocal costs ~6.3
                    # cyc/elem along the free dim, so spread S across 64
                    # partitions (DRAM bounce) before taking the reciprocal,
                    # then bounce back as a partition-broadcast row.
                    slot = h * 4 + js
                    rs = rr.tile([65, TS], f32, tag="rs")
                    nc.vector.tensor_copy(rs[srow:srow + 1, :], po[srow:srow + 1, :])
                    nc.sync.dma_start(out=scr[slot:slot + 1, :], in_=rs[srow:srow + 1, :])
                    rv = rr.tile([64, 8], f32, tag="rv")
                    nc.sync.dma_start(
                        out=rv,
                        in_=scr[slot:slot + 1, :].rearrange("a (p f) -> (a p) f", p=64),
                    )
                    rvr = rr.tile([64, 8], f32, tag="rvr")
                    nc.vector.reciprocal(rvr, rv)
                    nc.sync.dma_start(
                        out=scr2[slot:slot + 1, :].rearrange("a (p f) -> (a p) f", p=64),
                        in_=rvr,
                    )
                    rb = rr.tile([128, TS], f32, tag="rb")
                    sc = scr2[slot:slot + 1, :]
                    nc.gpsimd.dma_start(
                        out=rb[prow, :],
                        in_=bass.AP(tensor=sc.tensor, offset=sc.offset,
                                    ap=[[0, 64]] + list(sc.ap[1:])),
                    )
                    nc.vector.tensor_mul(yT_s[prow, h // 2, qsl], po[prow, :], rb[prow, :])

                # output projection deferred one q-super so the PE never
                # waits on the normalize chain (copy->DMA->recip->DMA->DMA->TT)
                oproj_js = js - 1 if js >= 1 else None
                if js == NTS - 1:
                    oproj_tbs = list(range(4 * (js - 1), 4 * js)) + list(range(4 * js, 4 * js + 4))
                elif js >= 1:
                    oproj_tbs = list(range(4 * (js - 1), 4 * js))
                else:
                    oproj_tbs = []
                for tb in oproj_tbs:
                    for cs in range(2):
                        py = psA.tile([128, 512], f32, tag="mm")
                        for jb in range(2):
                            nc.tensor.matmul(
                                py,
                                yT_s[:, jb, tb * 128:(tb + 1) * 128],
                                wp_s[:, jb, cs * 512:(cs + 1) * 512],
                                start=(jb == 0),
                                stop=(jb == 1),
                            )
                        ot = yo.tile([128, 512], f32, tag="ot")
                        nc.vector.tensor_copy(ot, py)
                        nc.sync.dma_start(
                            out=out[tb * 128:(tb + 1) * 128, cs * 512:(cs + 1) * 512],
                            in_=ot,
                        )

    _split_multiwaits(nc)
    return nc


def _rot_cols(w):
    """rotate_half as a column transform: out[:, d] = -w[:, d+32] (d<32), w[:, d-32] (d>=32)."""
    o = np.empty_like(w)
    o[..., :32] = -w[..., 32:64]
    o[..., 32:] = w[..., :32]
    return o


def _host_inputs(x, W_attn, b_attn, W_proj):
    f32 = np.float32
    inv = (1.0 / (10000.0 ** (np.arange(0, D, 2, dtype=f32) / f32(D)))).astype(f32)
    t = np.arange(T, dtype=f32)
    ang = np.outer(inv, t).astype(f32)            # [32, T]
    cos32, sin32 = np.cos(ang).astype(f32), np.sin(ang).astype(f32)
    cosb = np.tile(cos32, (4, 1))                  # [128, T], row p -> freq p%32
    sinb = np.tile(sin32, (4, 1))

    kk = np.arange(128)[:, None]
    qq = np.arange(TS)[None, :]
    masks = np.concatenate(
        [np.where(qq >= kk + 128 * rr_, f32(1), f32(0)) for rr_ in range(4)],
        axis=1,
    )                                              # [128, 4*TS] multiplicative

    import ml_dtypes

    bf16 = ml_dtypes.bfloat16
    p64 = np.zeros((D, D), dtype=f32)
    for d in range(32):
        p64[d, d + 32] = -1.0
        p64[d + 32, d] = 1.0
    pt2 = np.zeros((128, 128), dtype=f32)
    pt2[:64, :64] = p64.T
    pt2[64:, 64:] = p64.T
    pt2 = pt2.astype(bf16)

    xTs = [np.ascontiguousarray(x[b].T).astype(bf16) for b in range(B)]

    per_g = []
    for g in range(G):
        hs = [4 * g + j for j in range(G)]
        qcols = [W_attn[:, h * D:(h + 1) * D] for h in hs]
        kcols = [W_attn[:, C + h * D:C + (h + 1) * D] for h in hs]
        qb = [b_attn[h * D:(h + 1) * D] for h in hs]
        kb_ = [b_attn[C + h * D:C + (h + 1) * D] for h in hs]
        w1 = np.concatenate(
            [qcols[0], qcols[1], qcols[2], qcols[3], kcols[0], kcols[1], kcols[2], kcols[3]],
            axis=1,
        ).astype(bf16)                             # [C, 512]: [q01 q23 k01 k23]
        b1 = np.concatenate(qb + kb_).astype(f32).reshape(4, 128).T.copy()  # [128, 4]
        wv_ = W_attn[:, 2 * C + 256 * g:2 * C + 256 * (g + 1)].astype(bf16)
        wp_ = W_proj[256 * g:256 * (g + 1), :].astype(bf16)
        per_g.append((w1, b1, wv_, wp_))

    shared = dict(cosb=cosb, sinb=sinb, masks=masks.astype(bf16), pt2=pt2)
    in_maps = []
    for i in range(NCORES):
        b, g = i // 4, i % 4
        w1, b1, wv_, wp_ = per_g[g]
        in_maps.append(dict(xT=xTs[b], w1=w1, b1=b1, wv=wv_, wp=wp_, **shared))
    return in_maps


def kernel(x, W_attn, b_attn, W_proj, b_proj):
    from concourse.bass_utils import run_bass_kernel_spmd

    x = np.asarray(x, dtype=np.float32)
    W_attn = np.asarray(W_attn, dtype=np.float32)
    b_attn = np.asarray(b_attn, dtype=np.float32)
    W_proj = np.asarray(W_proj, dtype=np.float32)
    b_proj = np.asarray(b_proj, dtype=np.float32)

    if "nc" not in _cached:
        _cached["nc"] = _build()
    nc = _cached["nc"]

    in_maps = _host_inputs(x, W_attn, b_attn, W_proj)
    res = run_bass_kernel_spmd(nc, in_maps, core_ids=list(range(NCORES)))
    _cached["last_results"] = res

    const = (b_proj + b_attn[2 * C:] @ W_proj).astype(np.float32)
    y = np.empty((B, T, C), dtype=np.float32)
    for b in range(B):
        acc = res.results[4 * b]["out"].astype(np.float32).copy()
        for g in range(1, 4):
            acc += res.results[4 * b + g]["out"]
        y[b] = acc + const
    return y


# revision 28
# speedup vs baseline: 1.7395x; 1.0150x over previous
"""Causal self-attention (B=2, T=2048, C=1024, H=16, RoPE) on 8 TRN2 NeuronCores.

Sharding: core i handles batch b = i//4 and head group g = i%4 (4 heads each).
Each core computes q/k (transposed, RoPE'd), v, causal attention, and a partial
output projection; the host sums the 4 partials per batch element (tensor-
parallel unshard) and adds the constant term b_proj + b_v @ W_proj, which is
independent of x because softmax rows sum to 1.

Layout strategy (no on-chip transposes):
  - host passes x^T  [C, T]
  - q^T, k^T computed as (W^T x^T) with j (head*dim) on partitions
  - rotate_half(q) computed on-chip as P @ q^T (signed permutation matmul)
  - v computed in natural [t, j] layout, augmented with a ones column so the
    attention-value matmul also produces the softmax denominator
  - scores computed transposed: s^T[k, q] = k^T(d,k)^T . q^T(d,q); softmax
    normalization deferred until after AV (flash-style), no max subtraction
    (scores are ~N(0,1); exp is safe in fp32)
  - output projection consumes y^T directly: out[t, c] = y^T(j,t)^T . Wp(j,c)
Matmul operands are bf16 (full PE rate incl. 512-wide moving operands --
measured f32r falls to half rate there); softmax/RoPE arithmetic stays f32.
"""

import numpy as np

B, T, C, H, D = 2, 2048, 1024, 16, 64
G = 4           # heads per core
NCORES = 8
TS = 512        # t / q super-tile width
NT = T // 128   # 16 t-blocks
NTS = T // TS   # 4 t-supers
MASK_VAL = -1e5

_cached = {}


def _apply_workarounds():
    """This neuronxcc build rejects TPB instructions with >1 embedded sem wait.
    Patch the Tile drain and add a BIR pass splitting extra waits into
    standalone EventSemaphore instructions on the same (in-order) engine."""
    import concourse.tile as tile
    import concourse.mybir as mybir
    from concourse.vector_clock import ScopedClock

    if getattr(tile.TileContext, "_multiwait_patched", False):
        return

    def _drain_and_barrier(self, tick_clock, wait_clock):
        nc = self.nc
        probe = nc.sync.nop(nofuse=True)
        wait_clock.add_sem_waits(probe.ins, ScopedClock({None: tick_clock.global_clock}))
        si = probe.ins.sync_info
        waits = list(si.on_wait) if si and si.on_wait else []
        if si is not None:
            si.on_wait = []
        by_num = {h.num: h for h in self.sems.allocated().values()}
        for w in waits:
            nc.sync.wait_ge(by_num[w.id], w.wait_value)
        nc.sync.drain()
        nc.all_engine_barrier()
        popped = nc._tile_sem_poison_stack.pop()
        assert popped is self._sem_poison
        nc.clear_and_free_semaphores(list(self.sems.allocated().values()))
        nc.all_engine_barrier()

    tile.TileContext._drain_and_barrier = _drain_and_barrier
    tile.TileContext._multiwait_patched = True


def _split_multiwaits(nc, maxw=1):
    import concourse.mybir as mybir

    n = 0
    for f in nc.m.functions:
        for bb in f.blocks:
            insts = list(bb.instructions)
            out = []
            changed = False
            for inst in insts:
                si = inst.sync_info
                waits = list(si.on_wait) if si and si.on_wait else []
                if len(waits) > maxw:
                    for k, w in enumerate(waits[: len(waits) - maxw]):
                        out.append(
                            mybir.InstEventSemaphore(
                                name=f"{inst.name}-xw{k}",
                                engine=inst.engine,
                                ins=[],
                                outs=[],
                                sync_info=mybir.SyncInfo(on_wait=[w], on_update=[]),
                            )
                        )
                        n += 1
                    si.on_wait = waits[len(waits) - maxw :]
                    changed = True
                out.append(inst)
            if changed:
                bb.instructions.clear()
                for i in out:
                    bb.add_instruction(i)
    return n


def _build():
    import concourse.bass as bass
    import concourse.mybir as mybir
    import concourse.tile as tile

    _apply_workarounds()

    f32 = mybir.dt.float32
    bf16 = mybir.dt.bfloat16
    Exp = mybir.ActivationFunctionType.Exp
    Ident = mybir.ActivationFunctionType.Identity


    nc = bass.Bass()

    xT = nc.dram_tensor("xT", [C, T], bf16, kind="ExternalInput")
    w1 = nc.dram_tensor("w1", [C, 512], bf16, kind="ExternalInput")     # [q01 q23 k01 k23]
    b1 = nc.dram_tensor("b1", [128, 4], f32, kind="ExternalInput")
    wv = nc.dram_tensor("wv", [C, 256], bf16, kind="ExternalInput")
    wp = nc.dram_tensor("wp", [256, C], bf16, kind="ExternalInput")
    cosb = nc.dram_tensor("cosb", [128, T], f32, kind="ExternalInput")
    sinb = nc.dram_tensor("sinb", [128, T], f32, kind="ExternalInput")
    masks = nc.dram_tensor("masks", [128, 4 * TS], bf16, kind="ExternalInput")
    pt2 = nc.dram_tensor("pt2", [128, 128], bf16, kind="ExternalInput")  # rotate-half perm^T
    out = nc.dram_tensor("out", [T, C], f32, kind="ExternalOutput")
    scr = nc.dram_tensor("scr", [16, TS], f32)                          # S bounce
    scr2 = nc.dram_tensor("scr2", [16, TS], f32)                        # 1/S bounce

    with tile.TileContext(nc) as tc:
        with (
            tc.tile_pool(name="persist", bufs=1) as per,
            tc.tile_pool(name="xq", bufs=12) as xq,
            tc.tile_pool(name="qkr", bufs=4) as qkrp,
            tc.tile_pool(name="tmp", bufs=4) as tmp,
            tc.tile_pool(name="pp", bufs=6) as pp,
            tc.tile_pool(name="rr", bufs=4) as rr,
            tc.tile_pool(name="yo", bufs=4) as yo,
            tc.tile_pool(name="psA", bufs=4, space="PSUM") as psA,
            tc.tile_pool(name="psO", bufs=4, space="PSUM") as psO,
        ):
            # ---- persistent tiles ----
            w1_s = per.tile([128, 8, 512], bf16)
            wv_s = per.tile([128, 8, 256], bf16)
            wp_s = per.tile([128, 2, C], bf16)
            b1_s = per.tile([128, 4], f32)
            cos_s = per.tile([128, T], f32)
            sin_s = per.tile([128, T], f32)
            msk_s = per.tile([128, 4 * TS], bf16)
            pt2_s = per.tile([128, 128], bf16)
            qk_s = per.tile([128, 4, T], bf16)        # [q01' q23' k01' k23']
            # v storage per head pair:
            #   [v_even(0:64) | ones(64:66) | gap(66:97) | v_odd(97:161)]
            # The AV lhsT is a 128-wide window: even head -> cols 0..127, so y
            # lands in psum rows 0..63 with the denominator in row 64; odd
            # head -> cols 33..160, so y lands in rows 64..127 with the
            # denominator (ones col 65) in row 32. Engine ops require
            # 32-aligned partition starts, so denominator rows must be 32/64.
            v_s = per.tile([128, NT, 2, 161], bf16)
            yT_s = per.tile([128, 2, T], bf16)

            for cb in range(8):
                nc.sync.dma_start(out=w1_s[:, cb, :], in_=w1[cb * 128:(cb + 1) * 128, :])
                nc.sync.dma_start(out=wv_s[:, cb, :], in_=wv[cb * 128:(cb + 1) * 128, :])
            for jb in range(2):
                nc.sync.dma_start(out=wp_s[:, jb, :], in_=wp[jb * 128:(jb + 1) * 128, :])
            nc.sync.dma_start(out=b1_s, in_=b1[:])
            nc.sync.dma_start(out=cos_s, in_=cosb[:])
            nc.sync.dma_start(out=sin_s, in_=sinb[:])
            nc.sync.dma_start(out=msk_s, in_=masks[:])
            nc.sync.dma_start(out=pt2_s, in_=pt2[:])
            nc.vector.memset(v_s.rearrange("p a b c -> p (a b c)"), 1.0)

            # ---- attention for one q-super (called as soon as its
            # projections exist, so PE fills ACT-bound stretches with the
            # next t-super's projection matmuls) ----
            def do_attention(js):
                qsl = slice(js * TS, (js + 1) * TS)
                nkb = 4 * js + 4
                for h in range(G):
                    par = h % 2
                    prow = slice(par * 64, par * 64 + 64)
                    srow = 64 - 32 * par  # denominator row (32-aligned)
                    qT = qk_s[prow, h // 2, :]
                    kT = qk_s[prow, 2 + h // 2, :]

                    def v_win(kb, pair=h // 2, par=par):
                        # 128-wide lhsT window into the [v_even |1|1| v_odd] slot
                        return v_s[:, kb, pair, 33 * par:33 * par + 128]

                    po = psO.tile([128, 512], f32, tag="av")
                    pend = None
                    for kb in range(nkb):
                        pss = psA.tile([128, 512], f32, tag="mm")
                        nc.tensor.matmul(
                            pss,
                            kT[:, kb * 128:(kb + 1) * 128],
                            qT[:, qsl],
                            start=True,
                            stop=True,
                        )
                        pt = pp.tile([128, TS], bf16, tag="pt")
                        roff = kb - 4 * js
                        if roff >= 0:
                            # columns qq < 128*roff are fully masked: skip exp
                            # there and zero them; the partially-masked rest is
                            # zeroed multiplicatively after exp (bf16 SBUF TT
                            # is cheaper than an f32 PSUM-operand mask add).
                            w0 = 128 * roff
                            if w0:
                                nc.vector.memset(pt[:, :w0], 0.0)
                            nc.scalar.activation(pt[:, w0:], pss[:, w0:], Exp, scale=0.125)
                            nc.vector.tensor_mul(
                                pt[:, w0:], pt[:, w0:],
                                msk_s[:, roff * TS + w0:(roff + 1) * TS],
                            )
                        else:
                            nc.scalar.activation(pt, pss, Exp, scale=0.125)
                        if pend is not None:
                            nc.tensor.matmul(po, v_win(kb - 1), pend,
                                             start=(kb == 1), stop=False)
                        pend = pt
                    nc.tensor.matmul(po, v_win(nkb - 1), pend,
                                     start=False, stop=True)
                    # normalize y = po * (1/S). DVE InstReciprocal costs ~6.3
                    # cyc/elem along the free dim, so spread S across 64
                    # partitions (DRAM bounce) before taking the reciprocal,
                    # then bounce back as a partition-broadcast row.
                    slot = h * 4 + js
                    rs = rr.tile([65, TS], f32, tag="rs")
                    nc.vector.tensor_copy(rs[srow:srow + 1, :], po[srow:srow + 1, :])
                    nc.sync.dma_start(out=scr[slot:slot + 1, :], in_=rs[srow:srow + 1, :])
                    rv = rr.tile([64, 8], f32, tag="rv")
                    nc.sync.dma_start(
                        out=rv,
                        in_=scr[slot:slot + 1, :].rearrange("a (p f) -> (a p) f", p=64),
                    )
                    rvr = rr.tile([64, 8], f32, tag="rvr")
                    nc.vector.reciprocal(rvr, rv)
                    nc.sync.dma_start(
                        out=scr2[slot:slot + 1, :].rearrange("a (p f) -> (a p) f", p=64),
                        in_=rvr,
                    )
                    rb = rr.tile([128, TS], f32, tag="rb")
                    sc = scr2[slot:slot + 1, :]
                    nc.gpsimd.dma_start(
                        out=rb[prow, :],
                        in_=bass.AP(tensor=sc.tensor, offset=sc.offset,
                                    ap=[[0, 64]] + list(sc.ap[1:])),
                    )
                    nc.vector.tensor_mul(yT_s[prow, h // 2, qsl], po[prow, :], rb[prow, :])

                # output projection deferred one q-super so the PE never
                # waits on the normalize chain (copy->DMA->recip->DMA->DMA->TT)
                oproj_js = js - 1 if js >= 1 else None
                if js == NTS - 1:
                    oproj_tbs = list(range(4 * (js - 1), 4 * js)) + list(range(4 * js, 4 * js + 4))
                elif js >= 1:
                    oproj_tbs = list(range(4 * (js - 1), 4 * js))
                else:
                    oproj_tbs = []
                for tb in oproj_tbs:
                    for cs in range(2):
                        py = psA.tile([128, 512], f32, tag="mm")
                        for jb in range(2):
                            nc.tensor.matmul(
                                py,
                                yT_s[:, jb, tb * 128:(tb + 1) * 128],
                                wp_s[:, jb, cs * 512:(cs + 1) * 512],
                                start=(jb == 0),
                                stop=(jb == 1),
                            )
                        ot = yo.tile([128, 512], f32, tag="ot")
                        nc.vector.tensor_copy(ot, py)
                        nc.sync.dma_start(
                            out=out[tb * 128:(tb + 1) * 128, cs * 512:(cs + 1) * 512],
                            in_=ot,
                        )

            # ---- phase 1: q/k projection + RoPE, v projection ----
            for ts in range(NTS):
                tsl = slice(ts * TS, (ts + 1) * TS)
                xts = []
                for cb in range(8):
                    xt = xq.tile([128, TS], bf16, tag="xts")
                    nc.sync.dma_start(out=xt, in_=xT[cb * 128:(cb + 1) * 128, tsl])
                    xts.append(xt)
                for jb in range(4):
                    ps = psA.tile([128, 512], f32, tag="mm")
                    for cb in range(8):
                        nc.tensor.matmul(
                            ps,
                            w1_s[:, cb, jb * 128:(jb + 1) * 128],
                            xts[cb],
                            start=(cb == 0),
                            stop=(cb == 7),
                        )
                    qkr = qkrp.tile([128, TS], bf16, tag="qkr")
                    nc.scalar.activation(qkr, ps, Ident, bias=b1_s[:, jb:jb + 1], scale=1.0)
                    psr = psA.tile([128, 512], f32, tag="mm")
                    nc.tensor.matmul(psr, pt2_s, qkr, start=True, stop=True)
                    t1 = tmp.tile([128, TS], f32, tag="t1")
                    nc.vector.tensor_mul(t1, qkr, cos_s[:, tsl])
                    t2 = tmp.tile([128, TS], f32, tag="t2")
                    nc.vector.tensor_mul(t2, psr, sin_s[:, tsl])
                    nc.vector.tensor_add(qk_s[:, jb, tsl], t1, t2)
                for tb2 in range(4):
                    tb = ts * 4 + tb2
                    psv = psA.tile([128, 512], f32, tag="mm")
                    for cb in range(8):
                        nc.tensor.matmul(
                            psv[:, :256],
                            xts[cb][:, tb2 * 128:(tb2 + 1) * 128],
                            wv_s[:, cb, :],
                            start=(cb == 0),
                            stop=(cb == 7),
                        )
                    psv4 = psv[:, :256].rearrange("p (pr par d) -> p pr par d", par=2, d=D)
                    nc.vector.tensor_copy(v_s[:, tb, :, 0:64], psv4[:, :, 0, :])
                    nc.vector.tensor_copy(v_s[:, tb, :, 97:161], psv4[:, :, 1, :])

                do_attention(ts)

    _split_multiwaits(nc)
    return nc


def _rot_cols(w):
    """rotate_half as a column transform: out[:, d] = -w[:, d+32] (d<32), w[:, d-32] (d>=32)."""
    o = np.empty_like(w)
    o[..., :32] = -w[..., 32:64]
    o[..., 32:] = w[..., :32]
    return o


def _host_inputs(x, W_attn, b_attn, W_proj):
    f32 = np.float32
    inv = (1.0 / (10000.0 ** (np.arange(0, D, 2, dtype=f32) / f32(D)))).astype(f32)
    t = np.arange(T, dtype=f32)
    ang = np.outer(inv, t).astype(f32)            # [32, T]
    cos32, sin32 = np.cos(ang).astype(f32), np.sin(ang).astype(f32)
    cosb = np.tile(cos32, (4, 1))                  # [128, T], row p -> freq p%32
    sinb = np.tile(sin32, (4, 1))

    kk = np.arange(128)[:, None]
    qq = np.arange(TS)[None, :]
    masks = np.concatenate(
        [np.where(qq >= kk + 128 * rr_, f32(1), f32(0)) for rr_ in range(4)],
        axis=1,
    )                                              # [128, 4*TS] multiplicative

    import ml_dtypes

    bf16 = ml_dtypes.bfloat16
    p64 = np.zeros((D, D), dtype=f32)
    for d in range(32):
        p64[d, d + 32] = -1.0
        p64[d + 32, d] = 1.0
    pt2 = np.zeros((128, 128), dtype=f32)
    pt2[:64, :64] = p64.T
    pt2[64:, 64:] = p64.T
    pt2 = pt2.astype(bf16)

    xTs = [np.ascontiguousarray(x[b].T).astype(bf16) for b in range(B)]

    per_g = []
    for g in range(G):
        hs = [4 * g + j for j in range(G)]
        qcols = [W_attn[:, h * D:(h + 1) * D] for h in hs]
        kcols = [W_attn[:, C + h * D:C + (h + 1) * D] for h in hs]
        qb = [b_attn[h * D:(h + 1) * D] for h in hs]
        kb_ = [b_attn[C + h * D:C + (h + 1) * D] for h in hs]
        w1 = np.concatenate(
            [qcols[0], qcols[1], qcols[2], qcols[3], kcols[0], kcols[1], kcols[2], kcols[3]],
            axis=1,
        ).astype(bf16)                             # [C, 512]: [q01 q23 k01 k23]
        b1 = np.concatenate(qb + kb_).astype(f32).reshape(4, 128).T.copy()  # [128, 4]
        wv_ = W_attn[:, 2 * C + 256 * g:2 * C + 256 * (g + 1)].astype(bf16)
        wp_ = W_proj[256 * g:256 * (g + 1), :].astype(bf16)
        per_g.append((w1, b1, wv_, wp_))

    shared = dict(cosb=cosb, sinb=sinb, masks=masks.astype(bf16), pt2=pt2)
    in_maps = []
    for i in range(NCORES):
        b, g = i // 4, i % 4
        w1, b1, wv_, wp_ = per_g[g]
        in_maps.append(dict(xT=xTs[b], w1=w1, b1=b1, wv=wv_, wp=wp_, **shared))
    return in_maps


def kernel(x, W_attn, b_attn, W_proj, b_proj):
    from concourse.bass_utils import run_bass_kernel_spmd

    x = np.asarray(x, dtype=np.float32)
    W_attn = np.asarray(W_attn, dtype=np.float32)
    b_attn = np.asarray(b_attn, dtype=np.float32)
    W_proj = np.asarray(W_proj, dtype=np.float32)
    b_proj = np.asarray(b_proj, dtype=np.float32)

    if "nc" not in _cached:
        _cached["nc"] = _build()
    nc = _cached["nc"]

    in_maps = _host_inputs(x, W_attn, b_attn, W_proj)
    res = run_bass_kernel_spmd(nc, in_maps, core_ids=list(range(NCORES)))
    _cached["last_results"] = res

    const = (b_proj + b_attn[2 * C:] @ W_proj).astype(np.float32)
    y = np.empty((B, T, C), dtype=np.float32)
    for b in range(B):
        acc = res.results[4 * b]["out"].astype(np.float32).copy()
        for g in range(1, 4):
            acc += res.results[4 * b + g]["out"]
        y[b] = acc + const
    return y
